# revision 16
# baseline (speedup 1.0000x reference)
"""MoE gate (DeepSeek-style group-limited top-k routing) on 8 TRN2 NeuronCores.

Strategy (hardcoded for B=4, S=4096, H=4096, E=160, G=8, topk_group=3, top_k=6):
  - Token-parallel: 16384 tokens sharded 2048/core across 8 cores; the small
    [E,H] gate weight is replicated (pre-transposed + bf16 hi/lo split on host).
  - logits = x @ W.T computed as 3 bf16 matmuls per 128-contraction chunk
    (x_hi@W_hi + x_hi@W_lo + x_lo@W_hi) accumulated in PSUM -> fp32-accurate
    (logit abs err ~2e-6, measured: 1/16384 tokens flips one top-k index vs
    the fp32 jax reference).
  - x transposed on the fly with the DMA xbar transpose (bf16 2-byte dtype).
  - softmax on ACT (exp + row-sum fused), group-top3 + top6 via DVE Max8 /
    MaxIndex, per-core colsums of scores and selection mask via PE matmul
    with a ones vector; tiny final aux-loss reduction done host-side during
    the gather step.
"""

import os

import numpy as np
import ml_dtypes

# ---- problem constants (hardcoded; see module docstring) ----
B, S, H = 4, 4096, 4096
E, G = 160, 8
GSIZE = E // G  # 20
TOPK_GROUP, TOP_K = 3, 6
SCALE, ALPHA, M_CFG = 16.0, 0.001, 3
N_CORES = 8
T = B * S                    # 16384
T_CORE = T // N_CORES        # 2048
NCH = H // 128               # 32 contraction chunks
MACRO = 512                  # tokens per transpose macro-tile
N_MACRO = T_CORE // MACRO    # 4
NTILE = T_CORE // 128        # 16 token tiles per core

_CACHE = {}

# Diagnostic switch: 1 = host pre-transposes x, kernel uses plain DMA loads
HOST_TRANSPOSE = bool(int(os.environ.get("KBENCH_HOST_T", "0")))


def _build_nc():
    import concourse.bass as bass
    import concourse.bacc as bacc
    import concourse.mybir as mybir
    import concourse.tile as tile

    dt = mybir.dt
    f32 = dt.float32
    bf16 = dt.bfloat16
    X = mybir.AxisListType.X
    Alu = mybir.AluOpType
    Act = mybir.ActivationFunctionType

    nc = bacc.Bacc(
        "TRN2", target_bir_lowering=False, debug=False, num_devices=N_CORES
    )

    if HOST_TRANSPOSE:
        xhi_d = nc.dram_tensor("xhi", [H, T_CORE], bf16, kind="ExternalInput")
        xlo_d = nc.dram_tensor("xlo", [H, T_CORE], bf16, kind="ExternalInput")
    else:
        xhi_d = nc.dram_tensor("xhi", [T_CORE, H], bf16, kind="ExternalInput")
        xlo_d = nc.dram_tensor("xlo", [T_CORE, H], bf16, kind="ExternalInput")
    # packed transposed weight: [:, 0:E] = (W.T)_hi chunk, [:, E:2E] = (W.T)_lo
    wt_d = nc.dram_tensor("wt", [H, 2 * E], bf16, kind="ExternalInput")
    wout_d = nc.dram_tensor("wout", [T_CORE, TOP_K], f32, kind="ExternalOutput")
    iout_d = nc.dram_tensor("iout", [T_CORE, TOP_K], dt.uint32, kind="ExternalOutput")
    # [0, 0:E] = per-core colsum of softmax scores, [0, E:2E] = colsum of top-6
    # selection mask (expert counts)
    stats_d = nc.dram_tensor("stats", [1, 2 * E], f32, kind="ExternalOutput")

    xhi = xhi_d.ap()
    xlo = xlo_d.ap()
    wt = wt_d.ap()
    wout = wout_d.ap()
    iout = iout_d.ap()
    stats = stats_d.ap()

    with tile.TileContext(nc) as tc:
        with (
            tc.tile_pool(name="w", bufs=1) as wpool,
            tc.tile_pool(name="xT", bufs=2 * NCH) as xpool,
            tc.tile_pool(name="scores", bufs=3) as scpool,
            tc.tile_pool(name="small", bufs=3) as spool,
            tc.tile_pool(name="psum", bufs=4, space="PSUM") as psum_pool,
            tc.tile_pool(name="spsum", bufs=1, space="PSUM") as stats_pool,
        ):
            # gate weight, resident for the whole kernel: [128, chunk, 2E]
            wtile = wpool.tile([128, NCH, 2 * E], bf16)
            nc.sync.dma_start(wtile[:], wt.rearrange("(c p) e -> p c e", p=128))

            ones = wpool.tile([128, 1], f32)
            nc.vector.memset(ones[:], 1.0)

            stats_ps = stats_pool.tile([1, 2 * E], f32)

            # output staging, written across all 16 token tiles, DMA'd once
            wstage = wpool.tile([128, NTILE, TOP_K], f32)
            istage = wpool.tile([128, NTILE, 8], dt.uint32)

            gtile = 0
            for m in range(N_MACRO):
                tok0 = m * MACRO
                xhiT = []
                xloT = []
                for c in range(NCH):
                    th = xpool.tile([128, MACRO], bf16, tag="xhiT")
                    tl = xpool.tile([128, MACRO], bf16, tag="xloT")
                    if HOST_TRANSPOSE:
                        nc.sync.dma_start(
                            th[:], xhi[c * 128 : (c + 1) * 128, tok0 : tok0 + MACRO]
                        )
                        nc.scalar.dma_start(
                            tl[:], xlo[c * 128 : (c + 1) * 128, tok0 : tok0 + MACRO]
                        )
                    else:
                        nc.sync.dma_start_transpose(
                            th[:], xhi[tok0 : tok0 + MACRO, c * 128 : (c + 1) * 128]
                        )
                        nc.scalar.dma_start_transpose(
                            tl[:], xlo[tok0 : tok0 + MACRO, c * 128 : (c + 1) * 128]
                        )
                    xhiT.append(th)
                    xloT.append(tl)

                for tt in range(MACRO // 128):
                    t0 = tok0 + tt * 128
                    tsl = slice(tt * 128, (tt + 1) * 128)

                    ps = psum_pool.tile([128, E], f32)
                    for c in range(NCH):
                        nc.tensor.matmul(
                            out=ps[:],
                            lhsT=xhiT[c][:, tsl],
                            rhs=wtile[:, c, 0:E],
                            start=(c == 0),
                            stop=False,
                        )
                        nc.tensor.matmul(
                            out=ps[:],
                            lhsT=xhiT[c][:, tsl],
                            rhs=wtile[:, c, E : 2 * E],
                            start=False,
                            stop=False,
                        )
                        nc.tensor.matmul(
                            out=ps[:],
                            lhsT=xloT[c][:, tsl],
                            rhs=wtile[:, c, 0:E],
                            start=False,
                            stop=(c == NCH - 1),
                        )

                    # ---- softmax over E=160 ----
                    negmax = spool.tile([128, 1], f32, tag="negmax")
                    nc.vector.tensor_reduce(
                        negmax[:], ps[:], axis=X, op=Alu.max, negate=True
                    )
                    sc = scpool.tile([128, 2 * E], f32, tag="scores")
                    rowsum = spool.tile([128, 1], f32, tag="rowsum")
                    nc.scalar.activation(
                        sc[:, :E],
                        ps[:],
                        Act.Exp,
                        bias=negmax[:],
                        scale=1.0,
                        accum_out=rowsum[:],
                    )
                    recip = spool.tile([128, 1], f32, tag="recip")
                    nc.vector.reciprocal(recip[:], rowsum[:])
                    nc.vector.tensor_scalar_mul(sc[:, :E], sc[:, :E], recip[:])

                    # ---- group-limited top-3 groups ----
                    gmax = spool.tile([128, G], f32, tag="gmax")
                    nc.vector.tensor_reduce(
                        gmax[:],
                        sc[:, :E].rearrange("p (g j) -> p g j", g=G),
                        axis=X,
                        op=Alu.max,
                    )
                    g8 = spool.tile([128, 8], f32, tag="g8")
                    nc.vector.max(out=g8[:], in_=gmax[:])
                    gmask = spool.tile([128, G], f32, tag="gmask")
                    nc.vector.tensor_scalar(
                        gmask[:],
                        gmax[:],
                        g8[:, TOPK_GROUP - 1 : TOPK_GROUP],
                        None,
                        op0=Alu.is_ge,
                    )
                    msk = scpool.tile([128, E], f32, tag="masked")
                    nc.vector.tensor_tensor(
                        msk[:].rearrange("p (g j) -> p g j", g=G),
                        sc[:, :E].rearrange("p (g j) -> p g j", g=G),
                        gmask[:].unsqueeze(2).to_broadcast([128, G, GSIZE]),
                        op=Alu.mult,
                    )

                    # ---- top-6 experts ----
                    top8 = spool.tile([128, 8], f32, tag="top8")
                    nc.vector.max(out=top8[:], in_=msk[:])
                    nc.vector.max_index(
                        out=istage[:, gtile, :], in_max=top8[:], in_values=msk[:]
                    )
                    nc.vector.tensor_scalar_mul(
                        wstage[:, gtile, :], top8[:, :TOP_K], SCALE
                    )
                    # selection mask (counts) into sc[:, E:2E]
                    nc.vector.tensor_scalar(
                        sc[:, E : 2 * E],
                        msk[:],
                        top8[:, TOP_K - 1 : TOP_K],
                        None,
                        op0=Alu.is_ge,
                    )

                    # ---- per-core colsums (scores | selmask) via ones.T @ sc ----
                    nc.tensor.matmul(
                        out=stats_ps[:],
                        lhsT=ones[:],
                        rhs=sc[:],
                        start=(gtile == 0),
                        stop=(gtile == NTILE - 1),
                    )

                    gtile += 1

            nc.sync.dma_start(
                wout.rearrange("(n p) k -> p n k", p=128), wstage[:]
            )
            nc.sync.dma_start(
                iout.rearrange("(n p) k -> p n k", p=128), istage[:, :, :TOP_K]
            )
            stats_sb = spool.tile([1, 2 * E], f32, tag="stats_sb")
            nc.vector.tensor_copy(stats_sb[:], stats_ps[:])
            nc.gpsimd.dma_start(stats[:, :], stats_sb[:])

    nc.compile()
    return nc


def _get_nc():
    if "nc" not in _CACHE:
        _CACHE["nc"] = _build_nc()
    return _CACHE["nc"]


def kernel(hidden_states: np.ndarray, weight: np.ndarray):
    from concourse.bass_utils import run_bass_kernel_spmd

    nc = _get_nc()

    x = np.ascontiguousarray(
        np.asarray(hidden_states, dtype=np.float32).reshape(T, H)
    )
    bf = ml_dtypes.bfloat16
    xh = x.astype(bf)
    xl = (x - xh.astype(np.float32)).astype(bf)
    if HOST_TRANSPOSE:
        # per-core [H, T_CORE] layouts
        xh = np.ascontiguousarray(
            xh.reshape(N_CORES, T_CORE, H).transpose(0, 2, 1)
        )
        xl = np.ascontiguousarray(
            xl.reshape(N_CORES, T_CORE, H).transpose(0, 2, 1)
        )

    wtr = np.asarray(weight, dtype=np.float32).T  # [H, E]
    wh = wtr.astype(bf)
    wl = (wtr - wh.astype(np.float32)).astype(bf)
    wpack = np.ascontiguousarray(np.concatenate([wh, wl], axis=1))  # [H, 2E] bf16

    in_maps = []
    for c in range(N_CORES):
        if HOST_TRANSPOSE:
            xh_c, xl_c = xh[c], xl[c]
        else:
            s = slice(c * T_CORE, (c + 1) * T_CORE)
            xh_c = np.ascontiguousarray(xh[s])
            xl_c = np.ascontiguousarray(xl[s])
        in_maps.append({"xhi": xh_c, "xlo": xl_c, "wt": wpack})

    res = run_bass_kernel_spmd(
        nc, in_maps, core_ids=list(range(N_CORES)), trace=False
    )
    _CACHE["last_results"] = res

    outs = res.results
    topk_idx = np.concatenate(
        [o["iout"].astype(np.int32) for o in outs], axis=0
    )  # [T, 6]
    topk_weight = np.concatenate([o["wout"] for o in outs], axis=0)  # [T, 6]

    stats = np.stack([o["stats"][0] for o in outs], axis=0).astype(np.float64)
    # cores (2b, 2b+1) cover batch b
    sc_sum = stats[:, :E].reshape(B, 2, E).sum(axis=1)  # [B, E]
    counts = stats[:, E:].reshape(B, 2, E).sum(axis=1)  # [B, E]
    mean_scores = sc_sum / S
    ce = counts / (S * TOP_K / E)
    aux_expert = (ce * mean_scores).sum(axis=1).mean() * ALPHA
    ce_g = ce.reshape(B, G, -1).mean(axis=-1)
    ms_g = mean_scores.reshape(B, G, -1).mean(axis=-1)
    aux_device = (ce_g * ms_g).sum(axis=1).mean() * ALPHA
    ce2 = counts.reshape(B, G, -1).sum(axis=-1) / (M_CFG * S / G)
    aux_comm = (ce2 * ms_g).sum(axis=1).mean() * ALPHA
    aux_loss = np.float32(aux_expert + aux_device + aux_comm)

    return topk_idx, topk_weight, aux_loss


# revision 17
# speedup vs baseline: 1.1124x; 1.1124x over previous
"""MoE gate (DeepSeek-style group-limited top-k routing) on 8 TRN2 NeuronCores.

Strategy (hardcoded for B=4, S=4096, H=4096, E=160, G=8, topk_group=3, top_k=6):
  - Token-parallel: 16384 tokens sharded 2048/core across 8 cores; the small
    [E,H] gate weight is replicated.
  - logits = x @ W.T computed as 3 bf16 matmuls per 128-contraction chunk
    (x_hi@W_hi + x_hi@W_lo + x_lo@W_hi) accumulated in PSUM -> fp32-accurate
    (logit abs err ~2e-6; measured 1/16384 tokens flips one top-k index vs
    the fp32 jax reference).
  - Inputs are staged host-side into the exact partition-major tiled layout
    the kernel consumes ([macro, partition, chunk, token] for x hi/lo,
    [partition, chunk, 2E] for the packed W.T hi/lo), so every device DMA is
    a full-bandwidth contiguous read.
  - softmax on ACT (exp + row-sum fused), group-top3 + top6 via DVE Max8 /
    MaxIndex, per-core colsums of scores and the top-6 selection mask via PE
    matmul against a ones vector; the tiny O(B*E) aux-loss reduction runs
    host-side during the gather step.
"""

import os

import numpy as np
import ml_dtypes

# ---- problem constants (hardcoded; see module docstring) ----
B, S, H = 4, 4096, 4096
E, G = 160, 8
GSIZE = E // G  # 20
TOPK_GROUP, TOP_K = 3, 6
SCALE, ALPHA, M_CFG = 16.0, 0.001, 3
N_CORES = 8
T = B * S                    # 16384
T_CORE = T // N_CORES        # 2048
NCH = H // 128               # 32 contraction chunks
MACRO = 512                  # tokens per x macro-tile
N_MACRO = T_CORE // MACRO    # 4
NTILE = T_CORE // 128        # 16 token tiles per core
XL = NCH * MACRO             # free size of one x macro tile
DMA_SPLIT = 4                # sub-DMAs per macro tile (pipelining)

_CACHE = {}


def _build_nc():
    import concourse.bacc as bacc
    import concourse.mybir as mybir
    import concourse.tile as tile

    dt = mybir.dt
    f32 = dt.float32
    bf16 = dt.bfloat16
    X = mybir.AxisListType.X
    Alu = mybir.AluOpType
    Act = mybir.ActivationFunctionType

    nc = bacc.Bacc(
        "TRN2", target_bir_lowering=False, debug=False, num_devices=N_CORES
    )

    # x hi/lo in transposed tiled layout: [macro, p, chunk*MACRO + t]
    xhi_d = nc.dram_tensor("xhi", [N_MACRO, 128, XL], bf16, kind="ExternalInput")
    xlo_d = nc.dram_tensor("xlo", [N_MACRO, 128, XL], bf16, kind="ExternalInput")
    # packed transposed weight, partition-major: [p, chunk*(2E) + e]
    wt_d = nc.dram_tensor("wt", [128, NCH * 2 * E], bf16, kind="ExternalInput")
    wout_d = nc.dram_tensor("wout", [T_CORE, TOP_K], f32, kind="ExternalOutput")
    iout_d = nc.dram_tensor("iout", [T_CORE, TOP_K], dt.uint32, kind="ExternalOutput")
    # [0, 0:E] = per-core colsum of softmax scores, [0, E:2E] = expert counts
    stats_d = nc.dram_tensor("stats", [1, 2 * E], f32, kind="ExternalOutput")

    xhi = xhi_d.ap()
    xlo = xlo_d.ap()
    wt = wt_d.ap()
    wout = wout_d.ap()
    iout = iout_d.ap()
    stats = stats_d.ap()

    with tile.TileContext(nc) as tc:
        with (
            tc.tile_pool(name="w", bufs=1) as wpool,
            tc.tile_pool(name="xT", bufs=2) as xpool,
            tc.tile_pool(name="scores", bufs=3) as scpool,
            tc.tile_pool(name="small", bufs=3) as spool,
            tc.tile_pool(name="psum", bufs=4, space="PSUM") as psum_pool,
            tc.tile_pool(name="spsum", bufs=1, space="PSUM") as stats_pool,
        ):
            # gate weight, resident for the whole kernel: [128, chunk, 2E]
            wtile = wpool.tile([128, NCH * 2 * E], bf16)
            nc.sync.dma_start(wtile[:], wt)
            wview = wtile[:].rearrange("p (c e) -> p c e", c=NCH)

            ones = wpool.tile([128, 1], f32)
            nc.vector.memset(ones[:], 1.0)

            stats_ps = stats_pool.tile([1, 2 * E], f32)

            # output staging, written across all 16 token tiles, DMA'd once
            wstage = wpool.tile([128, NTILE, TOP_K], f32)
            istage = wpool.tile([128, NTILE, 8], dt.uint32)

            gtile = 0
            for m in range(N_MACRO):
                xh_t = xpool.tile([128, XL], bf16, tag="xhiT")
                xl_t = xpool.tile([128, XL], bf16, tag="xloT")
                step = XL // DMA_SPLIT
                for j in range(DMA_SPLIT):
                    sl = slice(j * step, (j + 1) * step)
                    nc.sync.dma_start(xh_t[:, sl], xhi[m, :, sl])
                    nc.scalar.dma_start(xl_t[:, sl], xlo[m, :, sl])

                for tt in range(MACRO // 128):
                    ps = psum_pool.tile([128, E], f32)
                    for c in range(NCH):
                        lsl = slice(c * MACRO + tt * 128, c * MACRO + (tt + 1) * 128)
                        nc.tensor.matmul(
                            out=ps[:],
                            lhsT=xh_t[:, lsl],
                            rhs=wview[:, c, 0:E],
                            start=(c == 0),
                            stop=False,
                        )
                        nc.tensor.matmul(
                            out=ps[:],
                            lhsT=xh_t[:, lsl],
                            rhs=wview[:, c, E : 2 * E],
                            start=False,
                            stop=False,
                        )
                        nc.tensor.matmul(
                            out=ps[:],
                            lhsT=xl_t[:, lsl],
                            rhs=wview[:, c, 0:E],
                            start=False,
                            stop=(c == NCH - 1),
                        )

                    # ---- softmax over E=160 ----
                    negmax = spool.tile([128, 1], f32, tag="negmax")
                    nc.vector.tensor_reduce(
                        negmax[:], ps[:], axis=X, op=Alu.max, negate=True
                    )
                    sc = scpool.tile([128, 2 * E], f32, tag="scores")
                    rowsum = spool.tile([128, 1], f32, tag="rowsum")
                    nc.scalar.activation(
                        sc[:, :E],
                        ps[:],
                        Act.Exp,
                        bias=negmax[:],
                        scale=1.0,
                        accum_out=rowsum[:],
                    )
                    recip = spool.tile([128, 1], f32, tag="recip")
                    nc.vector.reciprocal(recip[:], rowsum[:])
                    nc.vector.tensor_scalar_mul(sc[:, :E], sc[:, :E], recip[:])

                    # ---- group-limited top-3 groups ----
                    gmax = spool.tile([128, G], f32, tag="gmax")
                    nc.vector.tensor_reduce(
                        gmax[:],
                        sc[:, :E].rearrange("p (g j) -> p g j", g=G),
                        axis=X,
                        op=Alu.max,
                    )
                    g8 = spool.tile([128, 8], f32, tag="g8")
                    nc.vector.max(out=g8[:], in_=gmax[:])
                    gmask = spool.tile([128, G], f32, tag="gmask")
                    nc.vector.tensor_scalar(
                        gmask[:],
                        gmax[:],
                        g8[:, TOPK_GROUP - 1 : TOPK_GROUP],
                        None,
                        op0=Alu.is_ge,
                    )
                    msk = scpool.tile([128, E], f32, tag="masked")
                    nc.vector.tensor_tensor(
                        msk[:].rearrange("p (g j) -> p g j", g=G),
                        sc[:, :E].rearrange("p (g j) -> p g j", g=G),
                        gmask[:].unsqueeze(2).to_broadcast([128, G, GSIZE]),
                        op=Alu.mult,
                    )

                    # ---- top-6 experts ----
                    top8 = spool.tile([128, 8], f32, tag="top8")
                    nc.vector.max(out=top8[:], in_=msk[:])
                    nc.vector.max_index(
                        out=istage[:, gtile, :], in_max=top8[:], in_values=msk[:]
                    )
                    nc.vector.tensor_scalar_mul(
                        wstage[:, gtile, :], top8[:, :TOP_K], SCALE
                    )
                    # selection mask (counts) into sc[:, E:2E]
                    nc.vector.tensor_scalar(
                        sc[:, E : 2 * E],
                        msk[:],
                        top8[:, TOP_K - 1 : TOP_K],
                        None,
                        op0=Alu.is_ge,
                    )

                    # ---- per-core colsums (scores | selmask) via ones.T @ sc ----
                    nc.tensor.matmul(
                        out=stats_ps[:],
                        lhsT=ones[:],
                        rhs=sc[:],
                        start=(gtile == 0),
                        stop=(gtile == NTILE - 1),
                    )

                    gtile += 1

            nc.sync.dma_start(
                wout.rearrange("(n p) k -> p n k", p=128), wstage[:]
            )
            nc.sync.dma_start(
                iout.rearrange("(n p) k -> p n k", p=128), istage[:, :, :TOP_K]
            )
            stats_sb = spool.tile([1, 2 * E], f32, tag="stats_sb")
            nc.vector.tensor_copy(stats_sb[:], stats_ps[:])
            nc.gpsimd.dma_start(stats[:, :], stats_sb[:])

    nc.compile()
    return nc


def _get_nc():
    if "nc" not in _CACHE:
        _CACHE["nc"] = _build_nc()
    return _CACHE["nc"]


def _prep_inputs(hidden_states, weight):
    """Host-side staging: bf16 hi/lo split + partition-major tiled layouts."""
    x = np.ascontiguousarray(
        np.asarray(hidden_states, dtype=np.float32).reshape(T, H)
    )
    bf = ml_dtypes.bfloat16
    xh = x.astype(bf)
    xl = (x - xh.astype(np.float32)).astype(bf)

    def tile_x(a):
        # [N_CORES*T_CORE, H] -> per core [N_MACRO, 128, NCH*MACRO] with
        # layout [m, p, c*MACRO + t] = a[core*T_CORE + m*MACRO + t, c*128 + p]
        a = a.reshape(N_CORES, N_MACRO, MACRO, NCH, 128)
        a = a.transpose(0, 1, 4, 3, 2)  # [core, m, p, c, t]
        return np.ascontiguousarray(a.reshape(N_CORES, N_MACRO, 128, XL))

    xh_t = tile_x(xh)
    xl_t = tile_x(xl)

    wtr = np.asarray(weight, dtype=np.float32).T  # [H, E]
    wh = wtr.astype(bf)
    wl = (wtr - wh.astype(np.float32)).astype(bf)
    # [p, c, 2E]: [:, c, 0:E] = wh[c*128+p], [:, c, E:2E] = wl[c*128+p]
    wpk = np.concatenate(
        [wh.reshape(NCH, 128, E), wl.reshape(NCH, 128, E)], axis=2
    )  # [c, p, 2E]
    wpk = np.ascontiguousarray(
        wpk.transpose(1, 0, 2).reshape(128, NCH * 2 * E)
    )

    in_maps = []
    for c in range(N_CORES):
        in_maps.append({"xhi": xh_t[c], "xlo": xl_t[c], "wt": wpk})
    return in_maps


def _postprocess(outs):
    topk_idx = np.concatenate(
        [o["iout"].astype(np.int32) for o in outs], axis=0
    )  # [T, 6]
    topk_weight = np.concatenate([o["wout"] for o in outs], axis=0)  # [T, 6]

    stats = np.stack([o["stats"][0] for o in outs], axis=0).astype(np.float64)
    # cores (2b, 2b+1) cover batch b
    sc_sum = stats[:, :E].reshape(B, 2, E).sum(axis=1)  # [B, E]
    counts = stats[:, E:].reshape(B, 2, E).sum(axis=1)  # [B, E]
    mean_scores = sc_sum / S
    ce = counts / (S * TOP_K / E)
    aux_expert = (ce * mean_scores).sum(axis=1).mean() * ALPHA
    ce_g = ce.reshape(B, G, -1).mean(axis=-1)
    ms_g = mean_scores.reshape(B, G, -1).mean(axis=-1)
    aux_device = (ce_g * ms_g).sum(axis=1).mean() * ALPHA
    ce2 = counts.reshape(B, G, -1).sum(axis=-1) / (M_CFG * S / G)
    aux_comm = (ce2 * ms_g).sum(axis=1).mean() * ALPHA
    aux_loss = np.float32(aux_expert + aux_device + aux_comm)
    return topk_idx, topk_weight, aux_loss


def kernel(hidden_states: np.ndarray, weight: np.ndarray):
    from concourse.bass_utils import run_bass_kernel_spmd

    nc = _get_nc()
    in_maps = _prep_inputs(hidden_states, weight)
    res = run_bass_kernel_spmd(
        nc, in_maps, core_ids=list(range(N_CORES)), trace=False
    )
    _CACHE["last_results"] = res
    return _postprocess(res.results)
